# revision 1
# baseline (speedup 1.0000x reference)
"""DeepSet (phi -> masked sum pool -> rho) Trainium2 Bass kernel.

Problem (hardcoded):
  x:    [256, 1024, 64] f32, mask: [256, 1024] bool
  phi:  64->256 relu, 256->256 relu, 256->256 (+bias)
  pool: sum over N of mask * phi(x)
  rho:  256->256 relu, 256->256 relu, 256->128 (+bias)
  rows with no valid elements -> 0

Sharding: data-parallel over batch, 32 batches per core x 8 cores.
Weights replicated.

Device algorithm (per core, feature-major phi):
  - All matmul operands are fp16 (11-bit mantissa, ~5e-4 rounding). 2-byte
    weights lower to separate LDWEIGHTS+MATMUL, which avoids the walrus
    "Too many sync wait commands" limit that 4-byte (fp32/f32r) self-loading
    matmuls hit under Tile, and run the PE at 1 cycle/row.
  - mask is folded into x (x~ = m * x, per-partition scalar multiply on
    token-major tiles), phi runs unmasked on x~, and the pooled sum is
    corrected by inv_b * c where c = phi2(0) is captured ON DEVICE through
    the identical instruction path (exact cancellation).
  - pool is free: ScalarE activation(Relu, bias) accum_out add-reduces the
    output along the free dim into f32.
  - mm3 (last phi linear) is pulled after the pool (linearity), so it runs
    on 32 pooled vectors instead of 32768 tokens. The -inv*c and +cnt*b3
    corrections are applied in exact f32 on DVE using DMA-broadcast rows.
  - rho runs on [256, 32] feature-major tiles; final output is token-major
    so empty-set rows are zeroed with a per-partition multiply.
"""

import numpy as np

import concourse.bass as bass
import concourse.tile as tile
from concourse import mybir
from concourse.bass_utils import run_bass_kernel_spmd
from concourse.masks import make_identity

F32 = mybir.dt.float32
F16 = mybir.dt.float16
U8 = mybir.dt.uint8
AF = mybir.ActivationFunctionType
OP = mybir.AluOpType

NCORES = 8
XP_BUFS, XMP_BUFS, XTP_BUFS, H1RP_BUFS, LANDP_BUFS = 4, 16, 4, 4, 4
PS_XT_BUFS, PS_H1_BUFS, PS_H2_BUFS = 2, 2, 1
B, N, DIN, HID, DOUT = 256, 1024, 64, 256, 128
BPC = B // NCORES          # batches per core
CH = 512                   # tokens per chunk
NCH = N // CH              # chunks per batch
NBLK = CH // 128           # 128-token blocks per chunk


def _build_nc():
    nc = bass.Bass()

    x_d = nc.declare_dram_parameter("x", [BPC, N, DIN], F32, isOutput=False)
    mask_d = nc.declare_dram_parameter("mask", [BPC, N], U8, isOutput=False)
    w1_d = nc.declare_dram_parameter("w1", [DIN, HID], F16, isOutput=False)
    w2_d = nc.declare_dram_parameter("w2", [128, 512], F16, isOutput=False)
    w3_d = nc.declare_dram_parameter("w3", [128, 512], F16, isOutput=False)
    rw1_d = nc.declare_dram_parameter("rw1", [128, 512], F16, isOutput=False)
    rw2_d = nc.declare_dram_parameter("rw2", [128, 512], F16, isOutput=False)
    rw3_d = nc.declare_dram_parameter("rw3", [128, 256], F16, isOutput=False)
    # bias_block cols: 0,1=b1 halves  2,3=b2  4,5=b3  6,7=rb1  8,9=rb2 (f32)
    bias_d = nc.declare_dram_parameter("bias_block", [128, 16], F32, isOutput=False)
    rb3_d = nc.declare_dram_parameter("rb3_row", [1, 128], F16, isOutput=False)
    out_d = nc.declare_dram_parameter("out", [BPC, DOUT], F32, isOutput=True)

    with tile.TileContext(nc) as tc:
        with tc.tile_pool(name="singles", bufs=1) as singles:
            # ---- constants / weights ----
            w1_sb = singles.tile([DIN, HID], F16)
            nc.sync.dma_start(out=w1_sb, in_=w1_d[:, :])
            w2_sb = singles.tile([128, 512], F16)
            nc.sync.dma_start(out=w2_sb, in_=w2_d[:, :])
            w3_sb = singles.tile([128, 512], F16)
            nc.sync.dma_start(out=w3_sb, in_=w3_d[:, :])
            rw1_sb = singles.tile([128, 512], F16)
            nc.sync.dma_start(out=rw1_sb, in_=rw1_d[:, :])
            rw2_sb = singles.tile([128, 512], F16)
            nc.sync.dma_start(out=rw2_sb, in_=rw2_d[:, :])
            rw3_sb = singles.tile([128, 256], F16)
            nc.sync.dma_start(out=rw3_sb, in_=rw3_d[:, :])
            bias_sb = singles.tile([128, 16], F32)
            nc.sync.dma_start(out=bias_sb, in_=bias_d[:, :])
            rb3_sb = singles.tile([1, 128], F16)
            nc.sync.dma_start(out=rb3_sb, in_=rb3_d[:, :])

            ident = singles.tile([128, 128], F16)
            make_identity(nc, ident[:, :])
            ones_col = singles.tile([128, 1], F16)
            nc.vector.memset(ones_col[:, :], 1.0)
            ones_row = singles.tile([1, 32], F16)
            nc.vector.memset(ones_row[:, :], 1.0)

            # ---- mask prep: mT_sb[p, j*32+b] = mask[b, j*128+p] ----
            mask_u8 = singles.tile([BPC, N], U8)
            nc.sync.dma_start(out=mask_u8, in_=mask_d[:, :])
            mask_f = singles.tile([BPC, N], F16)
            nc.vector.tensor_copy(mask_f[:, :], mask_u8[:, :])

            mT_sb = singles.tile([128, 256], F32)   # scalar1 operand must be f32
            mT16_sb = singles.tile([128, 256], F16)  # fp16 copy for cnt matmuls
            cnt_sb = singles.tile([32, 1], F32)
            cnt16_sb = singles.tile([32, 1], F16)
            cnt_row = singles.tile([1, 32], F32)
            inv_row = singles.tile([1, 32], F32)

            with tc.tile_pool(name="prep_ps", bufs=1, space="PSUM") as pps:
                mT_ps = pps.tile([128, 256], F16)
                for j in range(8):
                    nc.tensor.transpose(
                        mT_ps[:, j * 32:(j + 1) * 32],
                        mask_f[:, j * 128:(j + 1) * 128],
                        ident[0:BPC, 0:BPC],
                    )
                nc.scalar.copy(mT_sb[:, :], mT_ps[:, :])
                nc.vector.tensor_copy(mT16_sb[:, :], mT_ps[:, :])
                cnt_ps = pps.tile([32, 1], F32)
                for j in range(8):
                    nc.tensor.matmul(
                        cnt_ps[:, :],
                        lhsT=mT16_sb[:, j * 32:(j + 1) * 32],
                        rhs=ones_col[:, :],
                        start=(j == 0),
                        stop=(j == 7),
                    )
                nc.scalar.copy(cnt_sb[:, :], cnt_ps[:, :])
                nc.vector.tensor_copy(cnt16_sb[:, :], cnt_sb[:, :])
                cnt_row_ps = pps.tile([1, 32], F32)
                nc.tensor.matmul(
                    cnt_row_ps[:, :],
                    lhsT=cnt16_sb[:, :],
                    rhs=ident[0:32, 0:32],
                    start=True,
                    stop=True,
                )
                nc.scalar.copy(cnt_row[:, :], cnt_row_ps[:, :])
                # inv = N - cnt  (exact f32)
                nc.vector.tensor_scalar(
                    inv_row[:, :], cnt_row[:, :], -1.0, float(N), OP.mult, OP.add,
                )
                # broadcast [1,32] rows across 128 partitions via rank-1
                # matmul (cnt/inv are integers <= 1024, exact in fp16)
                inv16_row = singles.tile([1, 32], F16)
                cnt16_row = singles.tile([1, 32], F16)
                nc.vector.tensor_copy(inv16_row[:, :], inv_row[:, :])
                nc.vector.tensor_copy(cnt16_row[:, :], cnt_row[:, :])
                ones_r128 = singles.tile([1, 128], F16)
                nc.vector.memset(ones_r128[:, :], 1.0)
                inv_bc = singles.tile([128, 32], F32)
                cnt_bc = singles.tile([128, 32], F32)
                for src, dst in ((inv16_row, inv_bc), (cnt16_row, cnt_bc)):
                    bc_ps = pps.tile([128, 32], F32, tag="bc_ps", name="bc_ps")
                    nc.tensor.matmul(
                        bc_ps[:, :], lhsT=ones_r128[:, :], rhs=src[:, :],
                        start=True, stop=True,
                    )
                    nc.scalar.copy(dst[:, :], bc_ps[:, :])

            # ---- c_dev: phi of a zeroed token, identical op path ----
            czero = singles.tile([DIN, 1], F16)
            nc.vector.memset(czero[:, :], 0.0)
            h1z_sb = singles.tile([128, 2], F16)
            landz = singles.tile([128, 2], F16)
            c_acc = singles.tile([128, 2], F32)

            with tc.tile_pool(name="cdev_ps", bufs=1, space="PSUM") as cps:
                for j in range(2):
                    h1z_ps = cps.tile([128, 1], F32, tag=f"h1z{j}")
                    nc.tensor.matmul(
                        h1z_ps[:, :],
                        lhsT=w1_sb[:, j * 128:(j + 1) * 128],
                        rhs=czero[:, :],
                        start=True,
                        stop=True,
                    )
                    nc.vector.tensor_scalar(
                        h1z_sb[:, j:j + 1], h1z_ps[:, :],
                        bias_sb[:, j:j + 1], 0.0, OP.add, OP.max,
                    )
                for j in range(2):
                    h2z_ps = cps.tile([128, 1], F32, tag=f"h2z{j}")
                    for k in range(2):
                        nc.tensor.matmul(
                            h2z_ps[:, :],
                            lhsT=w2_sb[:, k * 256 + j * 128: k * 256 + (j + 1) * 128],
                            rhs=h1z_sb[:, k:k + 1],
                            start=(k == 0),
                            stop=(k == 1),
                        )
                    nc.scalar.activation(
                        landz[:, j:j + 1], h2z_ps[:, :], AF.Relu,
                        bias=bias_sb[:, 2 + j:3 + j],
                        accum_out=c_acc[:, j:j + 1],
                    )

            # ---- main loop ----
            s_raw = [singles.tile([128, BPC * NCH], F32, tag=f"sraw{j}", name=f"sraw{j}")
                     for j in range(2)]

            with (
                tc.tile_pool(name="xp", bufs=XP_BUFS) as xp,
                tc.tile_pool(name="xmp", bufs=XMP_BUFS) as xmp,
                tc.tile_pool(name="xTp", bufs=XTP_BUFS) as xTp,
                tc.tile_pool(name="h1rp", bufs=H1RP_BUFS) as h1rp,
                tc.tile_pool(name="landp", bufs=LANDP_BUFS) as landp,
                tc.tile_pool(name="ps_xT", bufs=PS_XT_BUFS, space="PSUM") as ps_xT,
                tc.tile_pool(name="ps_h1", bufs=PS_H1_BUFS, space="PSUM") as ps_h1,
                tc.tile_pool(name="ps_h2", bufs=PS_H2_BUFS, space="PSUM") as ps_h2,
            ):
                for b in range(BPC):
                    for c in range(NCH):
                        ci = b * NCH + c
                        xT_ps = ps_xT.tile([DIN, CH], F16)
                        for k in range(NBLK):
                            jt = c * NBLK + k
                            xt = xp.tile([128, DIN], F32)
                            nc.sync.dma_start(
                                out=xt,
                                in_=x_d[b, c * CH + k * 128: c * CH + (k + 1) * 128, :],
                            )
                            xm = xmp.tile([128, DIN], F16)
                            nc.vector.tensor_scalar_mul(
                                xm[:, :], xt[:, :],
                                mT_sb[:, jt * 32 + b: jt * 32 + b + 1],
                            )
                            nc.tensor.transpose(
                                xT_ps[:, k * 128:(k + 1) * 128],
                                xm[:, :],
                                ident[:, :],
                            )
                        xT_sb = xTp.tile([DIN, CH], F16)
                        nc.scalar.copy(xT_sb[:, :], xT_ps[:, :])

                        h1_ps = [ps_h1.tile([128, CH], F32, tag=f"h1_{j}", name=f"h1_{j}")
                                 for j in range(2)]
                        for j in range(2):
                            nc.tensor.matmul(
                                h1_ps[j][:, :],
                                lhsT=w1_sb[:, j * 128:(j + 1) * 128],
                                rhs=xT_sb[:, :],
                                start=True,
                                stop=True,
                            )
                        h1r = [h1rp.tile([128, CH], F16, tag=f"h1r_{j}", name=f"h1r_{j}")
                               for j in range(2)]
                        for j in range(2):
                            nc.vector.tensor_scalar(
                                h1r[j][:, :], h1_ps[j][:, :],
                                bias_sb[:, j:j + 1], 0.0, OP.add, OP.max,
                            )
                        h2_ps = [ps_h2.tile([128, CH], F32, tag=f"h2_{j}", name=f"h2_{j}")
                                 for j in range(2)]
                        for j in range(2):
                            for k in range(2):
                                nc.tensor.matmul(
                                    h2_ps[j][:, :],
                                    lhsT=w2_sb[:, k * 256 + j * 128: k * 256 + (j + 1) * 128],
                                    rhs=h1r[k][:, :],
                                    start=(k == 0),
                                    stop=(k == 1),
                                )
                        for j in range(2):
                            land = landp.tile([128, CH], F16, tag=f"land_{j}", name=f"land_{j}")
                            nc.scalar.activation(
                                land[:, :], h2_ps[j][:, :], AF.Relu,
                                bias=bias_sb[:, 2 + j:3 + j],
                                accum_out=s_raw[j][:, ci:ci + 1],
                            )

            # ---- pooled correction (exact f32) + rho ----
            s_c16 = [singles.tile([128, BPC], F16, tag=f"sc16{j}", name=f"sc16{j}")
                     for j in range(2)]
            tmp_f32 = singles.tile([128, BPC], F32)
            s_f32 = singles.tile([128, BPC], F32)
            for j in range(2):
                sv = s_raw[j].rearrange("p (b c) -> p b c", c=NCH)
                nc.vector.tensor_tensor(
                    out=s_f32[:, :], in0=sv[:, :, 0], in1=sv[:, :, 1],
                    op=OP.add,
                )
                # s_corr = s - inv * c   (exact f32, per-partition scalar c)
                nc.vector.tensor_scalar_mul(
                    tmp_f32[:, :], inv_bc[:, :], c_acc[:, j:j + 1],
                )
                nc.vector.tensor_tensor(
                    out=s_c16[j][:, :], in0=s_f32[:, :], in1=tmp_f32[:, :],
                    op=OP.subtract,
                )

            with tc.tile_pool(name="rho_ps", bufs=1, space="PSUM") as rps:
                pooled_sb = [singles.tile([128, BPC], F16, tag=f"pool{j}", name=f"pool{j}")
                             for j in range(2)]
                for j in range(2):
                    p_ps = rps.tile([128, BPC], F32, tag=f"pps{j}")
                    for k in range(2):
                        nc.tensor.matmul(
                            p_ps[:, :],
                            lhsT=w3_sb[:, k * 256 + j * 128: k * 256 + (j + 1) * 128],
                            rhs=s_c16[k][:, :],
                            start=(k == 0),
                            stop=(k == 1),
                        )
                    # pooled += cnt * b3   (exact f32, then round to fp16)
                    nc.vector.tensor_scalar_mul(
                        tmp_f32[:, :], cnt_bc[:, :], bias_sb[:, 4 + j:5 + j],
                    )
                    nc.vector.tensor_tensor(
                        out=pooled_sb[j][:, :], in0=p_ps[:, :], in1=tmp_f32[:, :],
                        op=OP.add,
                    )

                r1_sb = [singles.tile([128, BPC], F16, tag=f"r1{j}", name=f"r1{j}")
                         for j in range(2)]
                for j in range(2):
                    r_ps = rps.tile([128, BPC], F32, tag=f"r1ps{j}")
                    for k in range(2):
                        nc.tensor.matmul(
                            r_ps[:, :],
                            lhsT=rw1_sb[:, k * 256 + j * 128: k * 256 + (j + 1) * 128],
                            rhs=pooled_sb[k][:, :],
                            start=(k == 0),
                            stop=(k == 1),
                        )
                    nc.scalar.activation(
                        r1_sb[j][:, :], r_ps[:, :], AF.Relu,
                        bias=bias_sb[:, 6 + j:7 + j],
                    )
                r2_sb = [singles.tile([128, BPC], F16, tag=f"r2{j}", name=f"r2{j}")
                         for j in range(2)]
                for j in range(2):
                    r_ps = rps.tile([128, BPC], F32, tag=f"r2ps{j}")
                    for k in range(2):
                        nc.tensor.matmul(
                            r_ps[:, :],
                            lhsT=rw2_sb[:, k * 256 + j * 128: k * 256 + (j + 1) * 128],
                            rhs=r1_sb[k][:, :],
                            start=(k == 0),
                            stop=(k == 1),
                        )
                    nc.scalar.activation(
                        r2_sb[j][:, :], r_ps[:, :], AF.Relu,
                        bias=bias_sb[:, 8 + j:9 + j],
                    )

                o_ps = rps.tile([BPC, DOUT], F32)
                for k in range(2):
                    nc.tensor.matmul(
                        o_ps[:, :],
                        lhsT=r2_sb[k][:, :],
                        rhs=rw3_sb[:, k * 128:(k + 1) * 128],
                        start=(k == 0),
                        stop=False,
                    )
                nc.tensor.matmul(
                    o_ps[:, :], lhsT=ones_row[:, 0:BPC], rhs=rb3_sb[:, :],
                    start=False, stop=True,
                )
                v_sb = singles.tile([32, 1], F32)
                nc.vector.tensor_scalar_min(v_sb[:, :], cnt_sb[:, :], 1.0)
                o_sb = singles.tile([BPC, DOUT], F32)
                nc.vector.tensor_scalar_mul(o_sb[:, :], o_ps[:, :], v_sb[:, :])
                nc.sync.dma_start(out=out_d[:, :], in_=o_sb[:, :])

    return nc


def _split_multi_waits(nc, max_waits=1):
    """Walrus codegen rejects instructions carrying more than one sync wait
    ("Too many sync wait commands"). Hoist excess waits onto single-wait
    EventSemaphore instructions inserted immediately before, on the same
    engine queue — semantically identical for in-order engine queues (the
    PE's LDWEIGHTS pull-ahead honors sem waits, and Ldweights carry at most
    one wait anyway)."""
    import copy as _copy

    counter = [0]

    def split(inst):
        si = inst.sync_info
        if si is None or si.on_wait is None or len(si.on_wait) <= max_waits:
            return [inst]
        if type(inst).__name__ == "InstEventSemaphore":
            keep = si.on_wait[:max_waits]
            extra = si.on_wait[max_waits:]
        else:
            extra = si.on_wait[:-max_waits]
            keep = si.on_wait[-max_waits:]
        pre = []
        for w in extra:
            counter[0] += 1
            es = mybir.InstEventSemaphore(
                name=f"ESW-{counter[0]}", engine=inst.engine, ins=[], outs=[],
            )
            es.sync_info = mybir.SyncInfo(on_wait=[w], on_update=[])
            pre.append(es)
        inst.sync_info = mybir.SyncInfo(
            on_wait=list(keep), on_update=list(si.on_update or [])
        )
        return pre + [inst]

    new_module = _copy.replace(nc.m, functions=[])
    for function in nc.m.functions:
        new_function = _copy.replace(function, blocks=[])
        new_function.set_allocations_from_list(function.allocations)
        for block in function.blocks:
            new_insts = []
            for inst in block.instructions:
                new_insts.extend(split(inst))
            new_function.blocks.append(
                _copy.replace(block, instructions=new_insts)
            )
        new_module.functions.append(new_function)
    nc.m = new_module
    return nc


_NC_CACHE = None


def _get_nc():
    global _NC_CACHE
    if _NC_CACHE is None:
        _NC_CACHE = _split_multi_waits(_build_nc())
    return _NC_CACHE


def _host_prep(inputs):
    """Weight layout re-arrangement + fp16 cast (weights only)."""
    def half_cols(v):  # [256] -> [128, 2]
        return np.ascontiguousarray(np.asarray(v, np.float32).reshape(2, 128).T)

    def pack(w):  # [256, X] f? -> [128, 2*X] fp16 with k-tiles side by side
        w = np.asarray(w, np.float32)
        k, x = w.shape
        return np.ascontiguousarray(
            w.reshape(2, 128, x).transpose(1, 0, 2).reshape(128, 2 * x)
        ).astype(np.float16)

    bias_block = np.zeros((128, 16), np.float32)
    bias_block[:, 0:2] = half_cols(inputs["pb1"])
    bias_block[:, 2:4] = half_cols(inputs["pb2"])
    bias_block[:, 4:6] = half_cols(inputs["pb3"])
    bias_block[:, 6:8] = half_cols(inputs["rb1"])
    bias_block[:, 8:10] = half_cols(inputs["rb2"])

    return {
        "w1": np.ascontiguousarray(np.asarray(inputs["pw1"], np.float32)).astype(np.float16),
        "w2": pack(inputs["pw2"]),
        "w3": pack(inputs["pw3"]),
        "rw1": pack(inputs["rw1"]),
        "rw2": pack(inputs["rw2"]),
        "rw3": np.ascontiguousarray(
            np.asarray(inputs["rw3"], np.float32)
            .reshape(2, 128, 128).transpose(1, 0, 2).reshape(128, 256)
        ).astype(np.float16),
        "bias_block": bias_block,
        "rb3_row": np.ascontiguousarray(
            np.asarray(inputs["rb3"], np.float32).reshape(1, 128)
        ).astype(np.float16),
    }


def kernel(**inputs) -> np.ndarray:
    nc = _get_nc()
    shared = _host_prep(inputs)
    x = np.asarray(inputs["x"], np.float32)
    mask = np.asarray(inputs["mask"]).astype(np.uint8)

    in_maps = []
    for core in range(NCORES):
        sl = slice(core * BPC, (core + 1) * BPC)
        m = dict(shared)
        m["x"] = np.ascontiguousarray(x[sl])
        m["mask"] = np.ascontiguousarray(mask[sl])
        in_maps.append(m)

    res = run_bass_kernel_spmd(nc, in_maps, core_ids=list(range(NCORES)))
    out = np.concatenate([res.results[i]["out"] for i in range(NCORES)], axis=0)
    return out.astype(np.float32)



# revision 13
# speedup vs baseline: 477.2759x; 477.2759x over previous
"""DeepSet (phi -> masked sum pool -> rho) Trainium2 Bass kernel, v2.

Problem (hardcoded):
  x:    [256, 1024, 64] f32, mask: [256, 1024] bool
  phi:  64->256 relu, 256->256 relu, 256->256 (+bias)
  pool: sum over N of mask * phi(x)
  rho:  256->256 relu, 256->256 relu, 256->128 (+bias)
  rows with no valid elements -> 0

Sharding: data-parallel over batch, 32 batches per core x 8 cores.
Weights replicated.

v2 changes vs v1 (which was sequencer-bound, all engine SEQs ~100%):
  - one big DMA per batch ([128, 8x64] f32) instead of 4 small ones/chunk
  - mask fold is ONE gpsimd (Pool engine) tensor_tensor per batch with a
    stride-0 broadcast AP over a host-side relayouted mask_mT (Pool was
    idle; mask+cast leaves DVE)
  - transposes packed 2 blocks wide ([128,128] lhsT) -> half the PE
    transpose instructions; mm1 uses partition-offset (0/64) operands on
    the packed layout (w1 replicated to partitions 64..127); the token
    order behind the pool permutes, which the sum doesn't care about
  - xT PSUM->SBUF copy moved from ScalarE to a DMA (engines were the
    bottleneck; DMA engines were 20% busy)
  - elementwise is balanced: h1 relu j0 on DVE / j1 on ScalarE, h2
    relu+bias+pool j0 on ScalarE (activation accum_out) / j1 on DVE
    (tensor_scalar accum_out)
  - `reps` build parameter re-runs the whole inference body N times for
    slope-based hardware timing (dispatch overhead on this axon setup is
    ~60-100ms, 1000x the kernel, so single-shot wall time is pure noise)

Numerics: identical to v1 — all matmul operands fp16, mask folded into x,
pool corrected by inv*c with c = phi2(0) captured on-device through the
same arithmetic (engine choice differs only in accumulation rounding).
"""

import numpy as np

import concourse.bass as bass
import concourse.tile as tile
from concourse import mybir
from concourse.bass_utils import run_bass_kernel_spmd
from concourse.masks import make_identity

F32 = mybir.dt.float32
F16 = mybir.dt.float16
U8 = mybir.dt.uint8
AF = mybir.ActivationFunctionType
OP = mybir.AluOpType

NCORES = 8
B, N, DIN, HID, DOUT = 256, 1024, 64, 256, 128
BPC = B // NCORES          # batches per core
NBLK = N // 128            # 128-token blocks per batch (8)
NCH = 2                    # pool-accumulator chunks per batch
CH = N // NCH              # tokens per chunk (512)


def _build_nc(reps=1):
    nc = bass.Bass()

    x_d = nc.declare_dram_parameter("x", [BPC, N, DIN], F32, isOutput=False)
    # mask_mT[p, b*8+j] = mask[b, j*128+p]  (batch-major cols, for bcast mult)
    mmT_d = nc.declare_dram_parameter("mask_mT", [128, BPC * NBLK], F32, isOutput=False)
    # mask_mTj[p, j*32+b] = mask[b, j*128+p]  (block-major cols, for cnt mms)
    mmTj_d = nc.declare_dram_parameter("mask_mTj", [128, BPC * NBLK], F16, isOutput=False)
    w1_d = nc.declare_dram_parameter("w1rep", [128, HID], F16, isOutput=False)
    w2_d = nc.declare_dram_parameter("w2", [128, 512], F16, isOutput=False)
    w3_d = nc.declare_dram_parameter("w3", [128, 512], F16, isOutput=False)
    rw1_d = nc.declare_dram_parameter("rw1", [128, 512], F16, isOutput=False)
    rw2_d = nc.declare_dram_parameter("rw2", [128, 512], F16, isOutput=False)
    rw3_d = nc.declare_dram_parameter("rw3", [128, 256], F16, isOutput=False)
    # bias_block cols: 0,1=b1 halves  2,3=b2  4,5=b3  6,7=rb1  8,9=rb2 (f32)
    bias_d = nc.declare_dram_parameter("bias_block", [128, 16], F32, isOutput=False)
    rb3_d = nc.declare_dram_parameter("rb3_row", [1, 128], F16, isOutput=False)
    out_d = nc.declare_dram_parameter("out", [BPC, DOUT], F32, isOutput=True)

    with tile.TileContext(nc) as tc:
        with tc.tile_pool(name="singles", bufs=1) as singles:
            # ---- constants / weights (one-time) ----
            w1_sb = singles.tile([128, HID], F16)
            nc.sync.dma_start(out=w1_sb, in_=w1_d[:, :])
            w2_sb = singles.tile([128, 512], F16)
            nc.sync.dma_start(out=w2_sb, in_=w2_d[:, :])
            w3_sb = singles.tile([128, 512], F16)
            nc.sync.dma_start(out=w3_sb, in_=w3_d[:, :])
            rw1_sb = singles.tile([128, 512], F16)
            nc.sync.dma_start(out=rw1_sb, in_=rw1_d[:, :])
            rw2_sb = singles.tile([128, 512], F16)
            nc.sync.dma_start(out=rw2_sb, in_=rw2_d[:, :])
            rw3_sb = singles.tile([128, 256], F16)
            nc.sync.dma_start(out=rw3_sb, in_=rw3_d[:, :])
            bias_sb = singles.tile([128, 16], F32)
            nc.sync.dma_start(out=bias_sb, in_=bias_d[:, :])
            rb3_sb = singles.tile([1, 128], F16)
            nc.sync.dma_start(out=rb3_sb, in_=rb3_d[:, :])
            mmT_sb = singles.tile([128, BPC * NBLK], F32)
            nc.sync.dma_start(out=mmT_sb, in_=mmT_d[:, :])
            mmTj_sb = singles.tile([128, BPC * NBLK], F16)
            nc.sync.dma_start(out=mmTj_sb, in_=mmTj_d[:, :])

            ident = singles.tile([128, 128], F16)
            make_identity(nc, ident[:, :])
            ones_col = singles.tile([128, 1], F16)
            nc.vector.memset(ones_col[:, :], 1.0)
            ones_row = singles.tile([1, 32], F16)
            nc.vector.memset(ones_row[:, :], 1.0)
            ones_r128 = singles.tile([1, 128], F16)
            nc.vector.memset(ones_r128[:, :], 1.0)
            zero_col = singles.tile([128, 1], F32)
            nc.vector.memset(zero_col[:, :], 0.0)

            # ---- c_dev: phi of a zeroed token, identical arithmetic ----
            czero = singles.tile([DIN, 1], F16)
            nc.vector.memset(czero[:, :], 0.0)
            h1z_sb = singles.tile([128, 2], F16)
            landz = singles.tile([128, 2], F16)
            c_acc = singles.tile([128, 2], F32)

            with tc.tile_pool(name="cdev_ps", bufs=1, space="PSUM") as cps:
                for j in range(2):
                    h1z_ps = cps.tile([128, 1], F32, tag=f"h1z{j}")
                    nc.tensor.matmul(
                        h1z_ps[:, :],
                        lhsT=w1_sb[0:DIN, j * 128:(j + 1) * 128],
                        rhs=czero[:, :],
                        start=True,
                        stop=True,
                    )
                    nc.vector.tensor_scalar(
                        h1z_sb[:, j:j + 1], h1z_ps[:, :],
                        bias_sb[:, j:j + 1], 0.0, OP.add, OP.max,
                    )
                for j in range(2):
                    h2z_ps = cps.tile([128, 1], F32, tag=f"h2z{j}")
                    for k in range(2):
                        nc.tensor.matmul(
                            h2z_ps[:, :],
                            lhsT=w2_sb[:, k * 256 + j * 128: k * 256 + (j + 1) * 128],
                            rhs=h1z_sb[:, k:k + 1],
                            start=(k == 0),
                            stop=(k == 1),
                        )
                    nc.scalar.activation(
                        landz[:, j:j + 1], h2z_ps[:, :], AF.Relu,
                        bias=bias_sb[:, 2 + j:3 + j],
                        accum_out=c_acc[:, j:j + 1],
                    )

            # ---- per-inference tiles (written every rep) ----
            cnt_sb = singles.tile([32, 1], F32)
            cnt16_sb = singles.tile([32, 1], F16)
            cnt_row = singles.tile([1, 32], F32)
            inv_row = singles.tile([1, 32], F32)
            inv16_row = singles.tile([1, 32], F16)
            cnt16_row = singles.tile([1, 32], F16)
            inv_bc = singles.tile([128, 32], F32)
            cnt_bc = singles.tile([128, 32], F32)
            s_raw = [singles.tile([128, BPC * NCH], F32, tag=f"sraw{j}", name=f"sraw{j}")
                     for j in range(2)]
            s_c16 = [singles.tile([128, BPC], F16, tag=f"sc16{j}", name=f"sc16{j}")
                     for j in range(2)]
            tmp_f32 = singles.tile([128, BPC], F32)
            s_f32 = singles.tile([128, BPC], F32)
            pooled_sb = [singles.tile([128, BPC], F16, tag=f"pool{j}", name=f"pool{j}")
                         for j in range(2)]
            r1_sb = [singles.tile([128, BPC], F16, tag=f"r1{j}", name=f"r1{j}")
                     for j in range(2)]
            r2_sb = [singles.tile([128, BPC], F16, tag=f"r2{j}", name=f"r2{j}")
                     for j in range(2)]
            v_sb = singles.tile([32, 1], F32)
            o_sb = singles.tile([BPC, DOUT], F32)

            for rep in range(reps):
                # ---- mask-derived prep (per inference) ----
                with tc.tile_pool(name="prep_ps", bufs=1, space="PSUM") as pps:
                    cnt_ps = pps.tile([32, 1], F32)
                    for j in range(NBLK):
                        nc.tensor.matmul(
                            cnt_ps[:, :],
                            lhsT=mmTj_sb[:, j * 32:(j + 1) * 32],
                            rhs=ones_col[:, :],
                            start=(j == 0),
                            stop=(j == NBLK - 1),
                        )
                    nc.scalar.copy(cnt_sb[:, :], cnt_ps[:, :])
                    nc.vector.tensor_copy(cnt16_sb[:, :], cnt_sb[:, :])
                    cnt_row_ps = pps.tile([1, 32], F32)
                    nc.tensor.matmul(
                        cnt_row_ps[:, :],
                        lhsT=cnt16_sb[:, :],
                        rhs=ident[0:32, 0:32],
                        start=True,
                        stop=True,
                    )
                    nc.scalar.copy(cnt_row[:, :], cnt_row_ps[:, :])
                    # inv = N - cnt  (exact f32)
                    nc.vector.tensor_scalar(
                        inv_row[:, :], cnt_row[:, :], -1.0, float(N), OP.mult, OP.add,
                    )
                    # broadcast [1,32] rows across 128 partitions via rank-1
                    # matmul (cnt/inv are integers <= 1024, exact in fp16)
                    nc.vector.tensor_copy(inv16_row[:, :], inv_row[:, :])
                    nc.vector.tensor_copy(cnt16_row[:, :], cnt_row[:, :])
                    for src, dst in ((inv16_row, inv_bc), (cnt16_row, cnt_bc)):
                        bc_ps = pps.tile([128, 32], F32, tag="bc_ps", name="bc_ps")
                        nc.tensor.matmul(
                            bc_ps[:, :], lhsT=ones_r128[:, :], rhs=src[:, :],
                            start=True, stop=True,
                        )
                        nc.scalar.copy(dst[:, :], bc_ps[:, :])

                # ---- main loop ----
                with (
                    tc.tile_pool(name="xp", bufs=3) as xp,
                    tc.tile_pool(name="xmp", bufs=3) as xmp,
                    tc.tile_pool(name="xTp", bufs=3) as xTp,
                    tc.tile_pool(name="h1rp", bufs=3) as h1rp,
                    tc.tile_pool(name="landp", bufs=3) as landp,
                    tc.tile_pool(name="ps_xT", bufs=2, space="PSUM") as ps_xT,
                    tc.tile_pool(name="ps_h1", bufs=1, space="PSUM") as ps_h1,
                    tc.tile_pool(name="ps_h2", bufs=1, space="PSUM") as ps_h2,
                ):
                    for b in range(BPC):
                        # one DMA: x[b] as [128, (4 k, 2 g, 64 feat)] f32,
                        # token block j = g*4 + k (g becomes the chunk id)
                        # partition p holds tokens {g*512 + 4p + i}: fully
                        # contiguous 1KB per partition per half (the token
                        # permutation is pool-invariant; mask_mT matches)
                        xt = xp.tile([128, NBLK * DIN], F32)
                        xt4 = xt.rearrange("p (k g d) -> p k g d", g=NCH, d=DIN)
                        for g in range(NCH):
                            nc.sync.dma_start(
                                out=xt4[:, :, g],
                                in_=x_d[b, g * CH:(g + 1) * CH]
                                    .rearrange("(p k) d -> p k d", p=128),
                            )
                        # mask fold + f16 cast on Pool engine, one op:
                        # xm[p, k, g, d] = x[p, k, g, d] * mask_mT[p, b*8+k*2+g]
                        xm = xmp.tile([128, NBLK * DIN], F16)
                        nc.gpsimd.tensor_tensor(
                            out=xm.rearrange("p (j d) -> p j d", d=DIN),
                            in0=xt.rearrange("p (j d) -> p j d", d=DIN),
                            in1=mmT_sb[:, b * NBLK:(b + 1) * NBLK, None]
                                .broadcast_to((128, NBLK, DIN)),
                            op=OP.mult,
                        )
                        # packed transposes: blocks (k, 4+k) stacked
                        # -> xT_ps[:, k*128:(k+1)*128] rows 0-63 = blk k
                        #    (chunk 0), rows 64-127 = blk 4+k (chunk 1).
                        # chunk c therefore lives on partition rows 64c..,
                        # and its token order permutes (the sum doesn't care)
                        xT_ps = ps_xT.tile([128, N // 2], F16)
                        for k in range(NBLK // 2):
                            nc.tensor.transpose(
                                xT_ps[:, k * 128:(k + 1) * 128],
                                xm[:, k * 128:(k + 1) * 128],
                                ident[:, :],
                            )
                        xT_sb = xTp.tile([128, N // 2], F16)
                        nc.scalar.copy(xT_sb[:, :], xT_ps[:, :])

                        # h1 merged per j across chunks: [128, (2 c, 512)]
                        # f32 = two PSUM banks; chunk c fills bank c
                        h1_ps = [ps_h1.tile([128, N], F32, tag=f"h1_{j}", name=f"h1_{j}")
                                 for j in range(2)]
                        for j in range(2):
                            for c in range(NCH):
                                nc.tensor.matmul(
                                    h1_ps[j][:, c * CH:(c + 1) * CH],
                                    lhsT=w1_sb[c * 64:(c + 1) * 64, j * 128:(j + 1) * 128],
                                    rhs=xT_sb[c * 64:(c + 1) * 64, :],
                                    start=True,
                                    stop=True,
                                )
                        h1r = [h1rp.tile([128, N], F16, tag=f"h1r_{j}", name=f"h1r_{j}")
                               for j in range(2)]
                        for j in range(2):
                            nc.scalar.activation(
                                h1r[j][:, :], h1_ps[j][:, :], AF.Relu,
                                bias=bias_sb[:, j:j + 1],
                            )
                        for c in range(NCH):
                            ci = b * NCH + c
                            h2_ps = [ps_h2.tile([128, CH], F32, tag=f"h2_{j}", name=f"h2_{j}")
                                     for j in range(2)]
                            for j in range(2):
                                for k in range(2):
                                    nc.tensor.matmul(
                                        h2_ps[j][:, :],
                                        lhsT=w2_sb[:, k * 256 + j * 128: k * 256 + (j + 1) * 128],
                                        rhs=h1r[k][:, c * CH:(c + 1) * CH],
                                        start=(k == 0),
                                        stop=(k == 1),
                                    )
                            for j in range(2):
                                # out = relu(h2 + b2), accum_out = sum(out):
                                # scalar_tensor_tensor is the one DVE op
                                # whose accumulator is a plain sum
                                land = landp.tile([128, CH], F16,
                                                  tag=f"land_{j}", name=f"land_{j}")
                                nc.vector.scalar_tensor_tensor(
                                    out=land[:, :], in0=h2_ps[j][:, :],
                                    scalar=bias_sb[:, 2 + j:3 + j],
                                    in1=zero_col[:, 0:1].broadcast_to((128, CH)),
                                    op0=OP.add, op1=OP.max,
                                    accum_out=s_raw[j][:, ci:ci + 1],
                                )

                # ---- pooled correction (exact f32) + rho ----
                for j in range(2):
                    sv = s_raw[j].rearrange("p (b c) -> p b c", c=NCH)
                    nc.vector.tensor_tensor(
                        out=s_f32[:, :], in0=sv[:, :, 0], in1=sv[:, :, 1],
                        op=OP.add,
                    )
                    # s_corr = s - inv * c   (exact f32, per-partition scalar c)
                    nc.vector.tensor_scalar_mul(
                        tmp_f32[:, :], inv_bc[:, :], c_acc[:, j:j + 1],
                    )
                    nc.vector.tensor_tensor(
                        out=s_c16[j][:, :], in0=s_f32[:, :], in1=tmp_f32[:, :],
                        op=OP.subtract,
                    )

                with tc.tile_pool(name="rho_ps", bufs=1, space="PSUM") as rps:
                    for j in range(2):
                        p_ps = rps.tile([128, BPC], F32, tag=f"pps{j}")
                        for k in range(2):
                            nc.tensor.matmul(
                                p_ps[:, :],
                                lhsT=w3_sb[:, k * 256 + j * 128: k * 256 + (j + 1) * 128],
                                rhs=s_c16[k][:, :],
                                start=(k == 0),
                                stop=(k == 1),
                            )
                        # pooled += cnt * b3   (exact f32, then round to fp16)
                        nc.vector.tensor_scalar_mul(
                            tmp_f32[:, :], cnt_bc[:, :], bias_sb[:, 4 + j:5 + j],
                        )
                        nc.vector.tensor_tensor(
                            out=pooled_sb[j][:, :], in0=p_ps[:, :], in1=tmp_f32[:, :],
                            op=OP.add,
                        )

                    for j in range(2):
                        r_ps = rps.tile([128, BPC], F32, tag=f"r1ps{j}")
                        for k in range(2):
                            nc.tensor.matmul(
                                r_ps[:, :],
                                lhsT=rw1_sb[:, k * 256 + j * 128: k * 256 + (j + 1) * 128],
                                rhs=pooled_sb[k][:, :],
                                start=(k == 0),
                                stop=(k == 1),
                            )
                        nc.scalar.activation(
                            r1_sb[j][:, :], r_ps[:, :], AF.Relu,
                            bias=bias_sb[:, 6 + j:7 + j],
                        )
                    for j in range(2):
                        r_ps = rps.tile([128, BPC], F32, tag=f"r2ps{j}")
                        for k in range(2):
                            nc.tensor.matmul(
                                r_ps[:, :],
                                lhsT=rw2_sb[:, k * 256 + j * 128: k * 256 + (j + 1) * 128],
                                rhs=r1_sb[k][:, :],
                                start=(k == 0),
                                stop=(k == 1),
                            )
                        nc.scalar.activation(
                            r2_sb[j][:, :], r_ps[:, :], AF.Relu,
                            bias=bias_sb[:, 8 + j:9 + j],
                        )

                    o_ps = rps.tile([BPC, DOUT], F32)
                    for k in range(2):
                        nc.tensor.matmul(
                            o_ps[:, :],
                            lhsT=r2_sb[k][:, :],
                            rhs=rw3_sb[:, k * 128:(k + 1) * 128],
                            start=(k == 0),
                            stop=False,
                        )
                    nc.tensor.matmul(
                        o_ps[:, :], lhsT=ones_row[:, 0:BPC], rhs=rb3_sb[:, :],
                        start=False, stop=True,
                    )
                    nc.vector.tensor_scalar_min(v_sb[:, :], cnt_sb[:, :], 1.0)
                    nc.vector.tensor_scalar_mul(o_sb[:, :], o_ps[:, :], v_sb[:, :])
                    nc.sync.dma_start(out=out_d[:, :], in_=o_sb[:, :])

    return nc


def _split_multi_waits(nc, max_waits=1):
    """Walrus codegen rejects instructions carrying more than one sync wait
    ("Too many sync wait commands"). Hoist excess waits onto single-wait
    EventSemaphore instructions inserted immediately before, on the same
    engine queue — semantically identical for in-order engine queues (the
    PE's LDWEIGHTS pull-ahead honors sem waits, and Ldweights carry at most
    one wait anyway)."""
    import copy as _copy

    counter = [0]

    def split(inst):
        si = inst.sync_info
        if si is None or si.on_wait is None or len(si.on_wait) <= max_waits:
            return [inst]
        if type(inst).__name__ == "InstEventSemaphore":
            keep = si.on_wait[:max_waits]
            extra = si.on_wait[max_waits:]
        else:
            extra = si.on_wait[:-max_waits]
            keep = si.on_wait[-max_waits:]
        pre = []
        for w in extra:
            counter[0] += 1
            es = mybir.InstEventSemaphore(
                name=f"ESW-{counter[0]}", engine=inst.engine, ins=[], outs=[],
            )
            es.sync_info = mybir.SyncInfo(on_wait=[w], on_update=[])
            pre.append(es)
        inst.sync_info = mybir.SyncInfo(
            on_wait=list(keep), on_update=list(si.on_update or [])
        )
        return pre + [inst]

    new_module = _copy.replace(nc.m, functions=[])
    for function in nc.m.functions:
        new_function = _copy.replace(function, blocks=[])
        new_function.set_allocations_from_list(function.allocations)
        for block in function.blocks:
            new_insts = []
            for inst in block.instructions:
                new_insts.extend(split(inst))
            new_function.blocks.append(
                _copy.replace(block, instructions=new_insts)
            )
        new_module.functions.append(new_function)
    nc.m = new_module
    return nc


_NC_CACHE = {}


def _get_nc(reps=1):
    if reps not in _NC_CACHE:
        _NC_CACHE[reps] = _split_multi_waits(_build_nc(reps))
    return _NC_CACHE[reps]


def _host_prep(inputs):
    """Weight layout re-arrangement + fp16 cast (weights only)."""
    def half_cols(v):  # [256] -> [128, 2]
        return np.ascontiguousarray(np.asarray(v, np.float32).reshape(2, 128).T)

    def pack(w):  # [256, X] f? -> [128, 2*X] fp16 with k-tiles side by side
        w = np.asarray(w, np.float32)
        k, x = w.shape
        return np.ascontiguousarray(
            w.reshape(2, 128, x).transpose(1, 0, 2).reshape(128, 2 * x)
        ).astype(np.float16)

    bias_block = np.zeros((128, 16), np.float32)
    bias_block[:, 0:2] = half_cols(inputs["pb1"])
    bias_block[:, 2:4] = half_cols(inputs["pb2"])
    bias_block[:, 4:6] = half_cols(inputs["pb3"])
    bias_block[:, 6:8] = half_cols(inputs["rb1"])
    bias_block[:, 8:10] = half_cols(inputs["rb2"])

    w1 = np.asarray(inputs["pw1"], np.float32).astype(np.float16)  # [64, 256]
    w1rep = np.ascontiguousarray(np.concatenate([w1, w1], axis=0))  # [128, 256]

    return {
        "w1rep": w1rep,
        "w2": pack(inputs["pw2"]),
        "w3": pack(inputs["pw3"]),
        "rw1": pack(inputs["rw1"]),
        "rw2": pack(inputs["rw2"]),
        "rw3": np.ascontiguousarray(
            np.asarray(inputs["rw3"], np.float32)
            .reshape(2, 128, 128).transpose(1, 0, 2).reshape(128, 256)
        ).astype(np.float16),
        "bias_block": bias_block,
        "rb3_row": np.ascontiguousarray(
            np.asarray(inputs["rb3"], np.float32).reshape(1, 128)
        ).astype(np.float16),
    }


def _mask_prep(mask_slice):
    """Layout transforms of the mask for one core: [BPC, N] bool ->
    mask_mT f32 [128, BPC*8] (batch-major cols, block order (k, g) with
    token block j = g*4 + k to match the interleaved x DMA) and
    mask_mTj f16 [128, 8*32] (block-major cols, for the cnt matmuls)."""
    # main-loop layout: partition p holds tokens g*512 + 4p + i, so the
    # mask col for (b, i, g) is mask[b, g*512 + 4p + i]
    m = np.asarray(mask_slice).astype(np.float32).reshape(BPC, NCH, 128, NBLK // NCH)
    # [p, b, i, g]
    m_mT = np.ascontiguousarray(
        m.transpose(2, 0, 3, 1).reshape(128, BPC * NBLK))
    m2 = np.asarray(mask_slice).astype(np.float32).reshape(BPC, NBLK, 128)
    m_mTj = np.ascontiguousarray(
        m2.transpose(2, 1, 0).reshape(128, NBLK * BPC)
    ).astype(np.float16)
    return m_mT, m_mTj


def kernel(**inputs) -> np.ndarray:
    nc = _get_nc()
    shared = _host_prep(inputs)
    x = np.asarray(inputs["x"], np.float32)
    mask = np.asarray(inputs["mask"])

    in_maps = []
    for core in range(NCORES):
        sl = slice(core * BPC, (core + 1) * BPC)
        m = dict(shared)
        m["x"] = np.ascontiguousarray(x[sl])
        m["mask_mT"], m["mask_mTj"] = _mask_prep(mask[sl])
        in_maps.append(m)

    res = run_bass_kernel_spmd(nc, in_maps, core_ids=list(range(NCORES)))
    out = np.concatenate([res.results[i]["out"] for i in range(NCORES)], axis=0)
    return out.astype(np.float32)


# revision 21
# speedup vs baseline: 694.2479x; 1.4546x over previous
"""DeepSet (phi -> masked sum pool -> rho) Trainium2 Bass kernel, v2.

Problem (hardcoded):
  x:    [256, 1024, 64] f32, mask: [256, 1024] bool
  phi:  64->256 relu, 256->256 relu, 256->256 (+bias)
  pool: sum over N of mask * phi(x)
  rho:  256->256 relu, 256->256 relu, 256->128 (+bias)
  rows with no valid elements -> 0

Sharding: data-parallel over batch, 32 batches per core x 8 cores.
Weights replicated.

v2 changes vs v1 (which was sequencer-bound, all engine SEQs ~100%):
  - one big DMA per batch ([128, 8x64] f32) instead of 4 small ones/chunk
  - mask fold is ONE gpsimd (Pool engine) tensor_tensor per batch with a
    stride-0 broadcast AP over a host-side relayouted mask_mT (Pool was
    idle; mask+cast leaves DVE)
  - transposes packed 2 blocks wide ([128,128] lhsT) -> half the PE
    transpose instructions; mm1 uses partition-offset (0/64) operands on
    the packed layout (w1 replicated to partitions 64..127); the token
    order behind the pool permutes, which the sum doesn't care about
  - chunk = partition-half of the packed xT (host interleaves token
    groups), so mm1 is 4 N=512 matmuls/batch and h1 lands in per-j
    [128,1024] two-bank PSUM tiles -> ONE big ScalarE relu per j
  - per-partition x rows are contiguous 1KB DMA reads (token order is
    pool-invariant, host lays mask_mT out to match)
  - elementwise balanced: xT copy + both h1 relus on ScalarE, all h2
    relu+bias+pool on DVE via scalar_tensor_tensor (its accum_out is a
    true sum; tensor_scalar's accumulator applies op1/scalar2 to the
    reduction instead of the output, so it cannot fuse relu+sum)
  - `reps` build parameter re-runs the whole inference body N times for
    slope-based hardware timing (dispatch overhead on this axon setup is
    ~60-100ms, 1000x the kernel, so single-shot wall time is pure noise)

Numerics: identical to v1 — all matmul operands fp16, mask folded into x,
pool corrected by inv*c with c = phi2(0) captured on-device through the
same arithmetic (engine choice differs only in accumulation rounding).
"""

import numpy as np

import concourse.bass as bass
import concourse.tile as tile
from concourse import mybir
from concourse.bass_utils import run_bass_kernel_spmd
from concourse.masks import make_identity

F32 = mybir.dt.float32
F16 = mybir.dt.float16
U8 = mybir.dt.uint8
AF = mybir.ActivationFunctionType
OP = mybir.AluOpType

NCORES = 8
B, N, DIN, HID, DOUT = 256, 1024, 64, 256, 128
BPC = B // NCORES          # batches per core
NBLK = N // 128            # 128-token blocks per batch (8)
NCH = 2                    # pool-accumulator chunks per batch
CH = N // NCH              # tokens per chunk (512)


def _build_nc(reps=1):
    nc = bass.Bass()

    x_d = nc.declare_dram_parameter("x", [BPC, N, DIN], F32, isOutput=False)
    # mask_mT[p, b*8+j] = mask[b, j*128+p]  (batch-major cols, for bcast mult)
    mmT_d = nc.declare_dram_parameter("mask_mT", [128, BPC * NBLK], F32, isOutput=False)
    # mask_mTj[p, j*32+b] = mask[b, j*128+p]  (block-major cols, for cnt mms)
    mmTj_d = nc.declare_dram_parameter("mask_mTj", [128, BPC * NBLK], F16, isOutput=False)
    w1_d = nc.declare_dram_parameter("w1rep", [128, HID], F16, isOutput=False)
    w2_d = nc.declare_dram_parameter("w2", [128, 512], F16, isOutput=False)
    w3_d = nc.declare_dram_parameter("w3", [128, 512], F16, isOutput=False)
    rw1_d = nc.declare_dram_parameter("rw1", [128, 512], F16, isOutput=False)
    rw2_d = nc.declare_dram_parameter("rw2", [128, 512], F16, isOutput=False)
    rw3_d = nc.declare_dram_parameter("rw3", [128, 256], F16, isOutput=False)
    # bias_block cols: 0,1=b1 halves  2,3=b2  4,5=b3  6,7=rb1  8,9=rb2 (f32)
    bias_d = nc.declare_dram_parameter("bias_block", [128, 16], F32, isOutput=False)
    rb3_d = nc.declare_dram_parameter("rb3_row", [1, 128], F16, isOutput=False)
    out_d = nc.declare_dram_parameter("out", [BPC, DOUT], F32, isOutput=True)

    with tile.TileContext(nc) as tc:
        with tc.tile_pool(name="singles", bufs=1) as singles:
            # ---- constants / weights (one-time) ----
            w1_sb = singles.tile([128, HID], F16)
            nc.sync.dma_start(out=w1_sb, in_=w1_d[:, :])
            w2_sb = singles.tile([128, 512], F16)
            nc.sync.dma_start(out=w2_sb, in_=w2_d[:, :])
            w3_sb = singles.tile([128, 512], F16)
            nc.sync.dma_start(out=w3_sb, in_=w3_d[:, :])
            rw1_sb = singles.tile([128, 512], F16)
            nc.sync.dma_start(out=rw1_sb, in_=rw1_d[:, :])
            rw2_sb = singles.tile([128, 512], F16)
            nc.sync.dma_start(out=rw2_sb, in_=rw2_d[:, :])
            rw3_sb = singles.tile([128, 256], F16)
            nc.sync.dma_start(out=rw3_sb, in_=rw3_d[:, :])
            bias_sb = singles.tile([128, 16], F32)
            nc.sync.dma_start(out=bias_sb, in_=bias_d[:, :])
            rb3_sb = singles.tile([1, 128], F16)
            nc.sync.dma_start(out=rb3_sb, in_=rb3_d[:, :])
            mmT_sb = singles.tile([128, BPC * NBLK], F32)
            nc.sync.dma_start(out=mmT_sb, in_=mmT_d[:, :])
            mmTj_sb = singles.tile([128, BPC * NBLK], F16)
            nc.sync.dma_start(out=mmTj_sb, in_=mmTj_d[:, :])

            ident = singles.tile([128, 128], F16)
            make_identity(nc, ident[:, :])
            ones_col = singles.tile([128, 1], F16)
            nc.vector.memset(ones_col[:, :], 1.0)
            ones_row = singles.tile([1, 32], F16)
            nc.vector.memset(ones_row[:, :], 1.0)
            ones_r128 = singles.tile([1, 128], F16)
            nc.vector.memset(ones_r128[:, :], 1.0)
            zero_col = singles.tile([128, 1], F32)
            nc.vector.memset(zero_col[:, :], 0.0)

            # ---- c_dev: phi of a zeroed token, identical arithmetic ----
            czero = singles.tile([DIN, 1], F16)
            nc.vector.memset(czero[:, :], 0.0)
            h1z_sb = singles.tile([128, 2], F16)
            landz = singles.tile([128, 2], F16)
            c_acc = singles.tile([128, 2], F32)

            with tc.tile_pool(name="cdev_ps", bufs=1, space="PSUM") as cps:
                for j in range(2):
                    h1z_ps = cps.tile([128, 1], F32, tag=f"h1z{j}")
                    nc.tensor.matmul(
                        h1z_ps[:, :],
                        lhsT=w1_sb[0:DIN, j * 128:(j + 1) * 128],
                        rhs=czero[:, :],
                        start=True,
                        stop=True,
                    )
                    nc.vector.tensor_scalar(
                        h1z_sb[:, j:j + 1], h1z_ps[:, :],
                        bias_sb[:, j:j + 1], 0.0, OP.add, OP.max,
                    )
                for j in range(2):
                    h2z_ps = cps.tile([128, 1], F32, tag=f"h2z{j}")
                    for k in range(2):
                        nc.tensor.matmul(
                            h2z_ps[:, :],
                            lhsT=w2_sb[:, k * 256 + j * 128: k * 256 + (j + 1) * 128],
                            rhs=h1z_sb[:, k:k + 1],
                            start=(k == 0),
                            stop=(k == 1),
                        )
                    nc.scalar.activation(
                        landz[:, j:j + 1], h2z_ps[:, :], AF.Relu,
                        bias=bias_sb[:, 2 + j:3 + j],
                        accum_out=c_acc[:, j:j + 1],
                    )

            # ---- per-inference tiles (written every rep) ----
            cnt_sb = singles.tile([32, 1], F32)
            cnt16_sb = singles.tile([32, 1], F16)
            cnt_row = singles.tile([1, 32], F32)
            inv_row = singles.tile([1, 32], F32)
            inv16_row = singles.tile([1, 32], F16)
            cnt16_row = singles.tile([1, 32], F16)
            inv_bc = singles.tile([128, 32], F32)
            cnt_bc = singles.tile([128, 32], F32)
            s_raw = [singles.tile([128, BPC * NCH], F32, tag=f"sraw{j}", name=f"sraw{j}")
                     for j in range(2)]
            s_c16 = [singles.tile([128, BPC], F16, tag=f"sc16{j}", name=f"sc16{j}")
                     for j in range(2)]
            tmp_f32 = singles.tile([128, BPC], F32)
            s_f32 = singles.tile([128, BPC], F32)
            pooled_sb = [singles.tile([128, BPC], F16, tag=f"pool{j}", name=f"pool{j}")
                         for j in range(2)]
            r1_sb = [singles.tile([128, BPC], F16, tag=f"r1{j}", name=f"r1{j}")
                     for j in range(2)]
            r2_sb = [singles.tile([128, BPC], F16, tag=f"r2{j}", name=f"r2{j}")
                     for j in range(2)]
            v_sb = singles.tile([32, 1], F32)
            o_sb = singles.tile([BPC, DOUT], F32)

            for rep in range(reps):
                # ---- mask-derived prep (per inference) ----
                with tc.tile_pool(name="prep_ps", bufs=1, space="PSUM") as pps:
                    cnt_ps = pps.tile([32, 1], F32, tag="cnt_ps")
                    for j in range(NBLK):
                        nc.tensor.matmul(
                            cnt_ps[:, :],
                            lhsT=mmTj_sb[:, j * 32:(j + 1) * 32],
                            rhs=ones_col[:, :],
                            start=(j == 0),
                            stop=(j == NBLK - 1),
                        )
                    nc.scalar.copy(cnt_sb[:, :], cnt_ps[:, :])
                    nc.vector.tensor_copy(cnt16_sb[:, :], cnt_sb[:, :])
                    cnt_row_ps = pps.tile([1, 32], F32, tag="cnt_row_ps")
                    nc.tensor.matmul(
                        cnt_row_ps[:, :],
                        lhsT=cnt16_sb[:, :],
                        rhs=ident[0:32, 0:32],
                        start=True,
                        stop=True,
                    )
                    nc.scalar.copy(cnt_row[:, :], cnt_row_ps[:, :])
                    # inv = N - cnt  (exact f32)
                    nc.vector.tensor_scalar(
                        inv_row[:, :], cnt_row[:, :], -1.0, float(N), OP.mult, OP.add,
                    )
                    # broadcast [1,32] rows across 128 partitions via rank-1
                    # matmul (cnt/inv are integers <= 1024, exact in fp16)
                    nc.vector.tensor_copy(inv16_row[:, :], inv_row[:, :])
                    nc.vector.tensor_copy(cnt16_row[:, :], cnt_row[:, :])
                    for src, dst in ((inv16_row, inv_bc), (cnt16_row, cnt_bc)):
                        bc_ps = pps.tile([128, 32], F32, tag="bc_ps", name="bc_ps")
                        nc.tensor.matmul(
                            bc_ps[:, :], lhsT=ones_r128[:, :], rhs=src[:, :],
                            start=True, stop=True,
                        )
                        nc.scalar.copy(dst[:, :], bc_ps[:, :])

                # ---- main loop ----
                with (
                    tc.tile_pool(name="xp", bufs=3) as xp,
                    tc.tile_pool(name="xmp", bufs=3) as xmp,
                    tc.tile_pool(name="xTp", bufs=3) as xTp,
                    tc.tile_pool(name="h1rp", bufs=3) as h1rp,
                    tc.tile_pool(name="landp", bufs=3) as landp,
                    tc.tile_pool(name="ps_xT", bufs=2, space="PSUM") as ps_xT,
                    tc.tile_pool(name="ps_h1", bufs=1, space="PSUM") as ps_h1,
                    tc.tile_pool(name="ps_h2", bufs=1, space="PSUM") as ps_h2,
                ):
                    for b in range(BPC):
                        # one DMA: x[b] as [128, (4 k, 2 g, 64 feat)] f32,
                        # token block j = g*4 + k (g becomes the chunk id)
                        # partition p holds tokens {g*512 + 4p + i}: fully
                        # contiguous 1KB per partition per half (the token
                        # permutation is pool-invariant; mask_mT matches)
                        xt = xp.tile([128, NBLK * DIN], F32)
                        xt4 = xt.rearrange("p (k g d) -> p k g d", g=NCH, d=DIN)
                        for g in range(NCH):
                            nc.sync.dma_start(
                                out=xt4[:, :, g],
                                in_=x_d[b, g * CH:(g + 1) * CH]
                                    .rearrange("(p k) d -> p k d", p=128),
                            )
                        # mask fold + f16 cast on Pool engine, one op:
                        # xm[p, k, g, d] = x[p, k, g, d] * mask_mT[p, b*8+k*2+g]
                        xm = xmp.tile([128, NBLK * DIN], F16)
                        nc.gpsimd.tensor_tensor(
                            out=xm.rearrange("p (j d) -> p j d", d=DIN),
                            in0=xt.rearrange("p (j d) -> p j d", d=DIN),
                            in1=mmT_sb[:, b * NBLK:(b + 1) * NBLK, None]
                                .broadcast_to((128, NBLK, DIN)),
                            op=OP.mult,
                        )
                        # packed transposes: blocks (k, 4+k) stacked
                        # -> xT_ps[:, k*128:(k+1)*128] rows 0-63 = blk k
                        #    (chunk 0), rows 64-127 = blk 4+k (chunk 1).
                        # chunk c therefore lives on partition rows 64c..,
                        # and its token order permutes (the sum doesn't care)
                        xT_ps = ps_xT.tile([128, N // 2], F16)
                        for k in range(NBLK // 2):
                            nc.tensor.transpose(
                                xT_ps[:, k * 128:(k + 1) * 128],
                                xm[:, k * 128:(k + 1) * 128],
                                ident[:, :],
                            )
                        xT_sb = xTp.tile([128, N // 2], F16)
                        nc.scalar.copy(xT_sb[:, :], xT_ps[:, :])

                        # h1 merged per j across chunks: [128, (2 c, 512)]
                        # f32 = two PSUM banks; chunk c fills bank c
                        h1_ps = [ps_h1.tile([128, N], F32, tag=f"h1_{j}", name=f"h1_{j}")
                                 for j in range(2)]
                        for j in range(2):
                            for c in range(NCH):
                                nc.tensor.matmul(
                                    h1_ps[j][:, c * CH:(c + 1) * CH],
                                    lhsT=w1_sb[c * 64:(c + 1) * 64, j * 128:(j + 1) * 128],
                                    rhs=xT_sb[c * 64:(c + 1) * 64, :],
                                    start=True,
                                    stop=True,
                                )
                        h1r = [h1rp.tile([128, N], F16, tag=f"h1r_{j}", name=f"h1r_{j}")
                               for j in range(2)]
                        for j in range(2):
                            nc.scalar.activation(
                                h1r[j][:, :], h1_ps[j][:, :], AF.Relu,
                                bias=bias_sb[:, j:j + 1],
                            )
                        for c in range(NCH):
                            ci = b * NCH + c
                            h2_ps = [ps_h2.tile([128, CH], F32, tag=f"h2_{j}", name=f"h2_{j}")
                                     for j in range(2)]
                            for j in range(2):
                                for k in range(2):
                                    nc.tensor.matmul(
                                        h2_ps[j][:, :],
                                        lhsT=w2_sb[:, k * 256 + j * 128: k * 256 + (j + 1) * 128],
                                        rhs=h1r[k][:, c * CH:(c + 1) * CH],
                                        start=(k == 0),
                                        stop=(k == 1),
                                    )
                            for j in range(2):
                                # out = relu(h2 + b2), accum_out = sum(out):
                                # scalar_tensor_tensor is the one DVE op
                                # whose accumulator is a plain sum
                                land = landp.tile([128, CH], F16,
                                                  tag=f"land_{j}", name=f"land_{j}")
                                nc.vector.scalar_tensor_tensor(
                                    out=land[:, :], in0=h2_ps[j][:, :],
                                    scalar=bias_sb[:, 2 + j:3 + j],
                                    in1=zero_col[:, 0:1].broadcast_to((128, CH)),
                                    op0=OP.add, op1=OP.max,
                                    accum_out=s_raw[j][:, ci:ci + 1],
                                )

                # ---- pooled correction (exact f32) + rho ----
                for j in range(2):
                    sv = s_raw[j].rearrange("p (b c) -> p b c", c=NCH)
                    nc.vector.tensor_tensor(
                        out=s_f32[:, :], in0=sv[:, :, 0], in1=sv[:, :, 1],
                        op=OP.add,
                    )
                    # s_corr = s - inv * c   (exact f32, per-partition scalar c)
                    nc.vector.tensor_scalar_mul(
                        tmp_f32[:, :], inv_bc[:, :], c_acc[:, j:j + 1],
                    )
                    nc.vector.tensor_tensor(
                        out=s_c16[j][:, :], in0=s_f32[:, :], in1=tmp_f32[:, :],
                        op=OP.subtract,
                    )

                with tc.tile_pool(name="rho_ps", bufs=1, space="PSUM") as rps:
                    for j in range(2):
                        p_ps = rps.tile([128, BPC], F32, tag=f"pps{j}")
                        for k in range(2):
                            nc.tensor.matmul(
                                p_ps[:, :],
                                lhsT=w3_sb[:, k * 256 + j * 128: k * 256 + (j + 1) * 128],
                                rhs=s_c16[k][:, :],
                                start=(k == 0),
                                stop=(k == 1),
                            )
                        # pooled += cnt * b3   (exact f32, then round to fp16)
                        nc.vector.tensor_scalar_mul(
                            tmp_f32[:, :], cnt_bc[:, :], bias_sb[:, 4 + j:5 + j],
                        )
                        nc.vector.tensor_tensor(
                            out=pooled_sb[j][:, :], in0=p_ps[:, :], in1=tmp_f32[:, :],
                            op=OP.add,
                        )

                    for j in range(2):
                        r_ps = rps.tile([128, BPC], F32, tag=f"r1ps{j}")
                        for k in range(2):
                            nc.tensor.matmul(
                                r_ps[:, :],
                                lhsT=rw1_sb[:, k * 256 + j * 128: k * 256 + (j + 1) * 128],
                                rhs=pooled_sb[k][:, :],
                                start=(k == 0),
                                stop=(k == 1),
                            )
                        nc.scalar.activation(
                            r1_sb[j][:, :], r_ps[:, :], AF.Relu,
                            bias=bias_sb[:, 6 + j:7 + j],
                        )
                    for j in range(2):
                        r_ps = rps.tile([128, BPC], F32, tag=f"r2ps{j}")
                        for k in range(2):
                            nc.tensor.matmul(
                                r_ps[:, :],
                                lhsT=rw2_sb[:, k * 256 + j * 128: k * 256 + (j + 1) * 128],
                                rhs=r1_sb[k][:, :],
                                start=(k == 0),
                                stop=(k == 1),
                            )
                        nc.scalar.activation(
                            r2_sb[j][:, :], r_ps[:, :], AF.Relu,
                            bias=bias_sb[:, 8 + j:9 + j],
                        )

                    o_ps = rps.tile([BPC, DOUT], F32, tag="o_ps")
                    for k in range(2):
                        nc.tensor.matmul(
                            o_ps[:, :],
                            lhsT=r2_sb[k][:, :],
                            rhs=rw3_sb[:, k * 128:(k + 1) * 128],
                            start=(k == 0),
                            stop=False,
                        )
                    nc.tensor.matmul(
                        o_ps[:, :], lhsT=ones_row[:, 0:BPC], rhs=rb3_sb[:, :],
                        start=False, stop=True,
                    )
                    nc.vector.tensor_scalar_min(v_sb[:, :], cnt_sb[:, :], 1.0)
                    nc.vector.tensor_scalar_mul(o_sb[:, :], o_ps[:, :], v_sb[:, :])
                    nc.sync.dma_start(out=out_d[:, :], in_=o_sb[:, :])

    return nc


def _split_multi_waits(nc, max_waits=1):
    """Walrus codegen rejects instructions carrying more than one sync wait
    ("Too many sync wait commands"). Hoist excess waits onto single-wait
    EventSemaphore instructions inserted immediately before, on the same
    engine queue — semantically identical for in-order engine queues (the
    PE's LDWEIGHTS pull-ahead honors sem waits, and Ldweights carry at most
    one wait anyway)."""
    import copy as _copy

    counter = [0]

    def split(inst):
        si = inst.sync_info
        if si is None or si.on_wait is None or len(si.on_wait) <= max_waits:
            return [inst]
        if type(inst).__name__ == "InstEventSemaphore":
            keep = si.on_wait[:max_waits]
            extra = si.on_wait[max_waits:]
        else:
            extra = si.on_wait[:-max_waits]
            keep = si.on_wait[-max_waits:]
        pre = []
        for w in extra:
            counter[0] += 1
            es = mybir.InstEventSemaphore(
                name=f"ESW-{counter[0]}", engine=inst.engine, ins=[], outs=[],
            )
            es.sync_info = mybir.SyncInfo(on_wait=[w], on_update=[])
            pre.append(es)
        inst.sync_info = mybir.SyncInfo(
            on_wait=list(keep), on_update=list(si.on_update or [])
        )
        return pre + [inst]

    new_module = _copy.replace(nc.m, functions=[])
    for function in nc.m.functions:
        new_function = _copy.replace(function, blocks=[])
        new_function.set_allocations_from_list(function.allocations)
        for block in function.blocks:
            new_insts = []
            for inst in block.instructions:
                new_insts.extend(split(inst))
            new_function.blocks.append(
                _copy.replace(block, instructions=new_insts)
            )
        new_module.functions.append(new_function)
    nc.m = new_module
    return nc


_NC_CACHE = {}


def _get_nc(reps=1):
    if reps not in _NC_CACHE:
        _NC_CACHE[reps] = _split_multi_waits(_build_nc(reps))
    return _NC_CACHE[reps]


def _host_prep(inputs):
    """Weight layout re-arrangement + fp16 cast (weights only)."""
    def half_cols(v):  # [256] -> [128, 2]
        return np.ascontiguousarray(np.asarray(v, np.float32).reshape(2, 128).T)

    def pack(w):  # [256, X] f? -> [128, 2*X] fp16 with k-tiles side by side
        w = np.asarray(w, np.float32)
        k, x = w.shape
        return np.ascontiguousarray(
            w.reshape(2, 128, x).transpose(1, 0, 2).reshape(128, 2 * x)
        ).astype(np.float16)

    bias_block = np.zeros((128, 16), np.float32)
    bias_block[:, 0:2] = half_cols(inputs["pb1"])
    bias_block[:, 2:4] = half_cols(inputs["pb2"])
    bias_block[:, 4:6] = half_cols(inputs["pb3"])
    bias_block[:, 6:8] = half_cols(inputs["rb1"])
    bias_block[:, 8:10] = half_cols(inputs["rb2"])

    w1 = np.asarray(inputs["pw1"], np.float32).astype(np.float16)  # [64, 256]
    w1rep = np.ascontiguousarray(np.concatenate([w1, w1], axis=0))  # [128, 256]

    return {
        "w1rep": w1rep,
        "w2": pack(inputs["pw2"]),
        "w3": pack(inputs["pw3"]),
        "rw1": pack(inputs["rw1"]),
        "rw2": pack(inputs["rw2"]),
        "rw3": np.ascontiguousarray(
            np.asarray(inputs["rw3"], np.float32)
            .reshape(2, 128, 128).transpose(1, 0, 2).reshape(128, 256)
        ).astype(np.float16),
        "bias_block": bias_block,
        "rb3_row": np.ascontiguousarray(
            np.asarray(inputs["rb3"], np.float32).reshape(1, 128)
        ).astype(np.float16),
    }


def _mask_prep(mask_slice):
    """Layout transforms of the mask for one core: [BPC, N] bool ->
    mask_mT f32 [128, BPC*8] (batch-major cols, block order (k, g) with
    token block j = g*4 + k to match the interleaved x DMA) and
    mask_mTj f16 [128, 8*32] (block-major cols, for the cnt matmuls)."""
    # main-loop layout: partition p holds tokens g*512 + 4p + i, so the
    # mask col for (b, i, g) is mask[b, g*512 + 4p + i]
    m = np.asarray(mask_slice).astype(np.float32).reshape(BPC, NCH, 128, NBLK // NCH)
    # [p, b, i, g]
    m_mT = np.ascontiguousarray(
        m.transpose(2, 0, 3, 1).reshape(128, BPC * NBLK))
    m2 = np.asarray(mask_slice).astype(np.float32).reshape(BPC, NBLK, 128)
    m_mTj = np.ascontiguousarray(
        m2.transpose(2, 1, 0).reshape(128, NBLK * BPC)
    ).astype(np.float16)
    return m_mT, m_mTj


def kernel(**inputs) -> np.ndarray:
    nc = _get_nc()
    shared = _host_prep(inputs)
    x = np.asarray(inputs["x"], np.float32)
    mask = np.asarray(inputs["mask"])

    in_maps = []
    for core in range(NCORES):
        sl = slice(core * BPC, (core + 1) * BPC)
        m = dict(shared)
        m["x"] = np.ascontiguousarray(x[sl])
        m["mask_mT"], m["mask_mTj"] = _mask_prep(mask[sl])
        in_maps.append(m)

    res = run_bass_kernel_spmd(nc, in_maps, core_ids=list(range(NCORES)))
    out = np.concatenate([res.results[i]["out"] for i in range(NCORES)], axis=0)
    return out.astype(np.float32)
